# revision 37
# baseline (speedup 1.0000x reference)
"""Trainium2 Bass kernel for nn_CAMD_9990093930844 (sparse_attention).

Math: the reference computes, per modality m,
    out_m[i, :] = Q[i] @ S_m(t1[i]) ,  S_m(t) = sum_{j: t2_m[j] <= t} K_m[j] (x) V_m[j]
and returns (sum_m out_m)[:, :2].  Only V[:, :2] matters, so this is
    out[i, v] = sum_m sum_{j: t2_m[j] <= t1[i]} (Q[i] . K_m[j]) * V_m[j, v],  v in {0, 1}

Both t1 and t2_m are sorted, so the rank deviation |p_m[i] - i| (p =
searchsorted) is bounded (~90 for this data).  Each 128-query chunk b
therefore only needs:
  - an unconditional prefix state over key chunks [0, b-1)  (PE chunk-sum accum)
  - a masked 128x384 local attention over key chunks {b-1, b, b+1}
No gather, no cumsum, no data-dependent control flow on device.

Sharding: 8 cores = 4 modalities x 2 query halves.  Host does the final
(tiny) sum over modalities + concat of halves.  Each core gets a 33-chunk
local key buffer (zero/sentinel-padded) so all 8 cores run one uniform SPMD
program.

Precision: MLP and score matmuls run as float32r (fast PE datapath,
~1e-4 relative).  Timestamps, the mask compare, V, prefix-state and output
accumulation stay exact fp32, so the time masking is bit-exact.
"""

import numpy as np

T = 4096
D = 66
M = 4
PC = 128                 # rows per chunk (partition dim)
QCH = 16                 # query chunks per core
KCH = 33                 # local key chunks per core
QW = QCH * PC            # 2048 queries per core
KW = KCH * PC            # 4224 local keys per core
TBIG = 1.0e30            # timestamp sentinel for padded keys (> any real t)


def _shard_host(x1, x2, x3, x4, wq, bq, wk, bk):
    """Build the 8 per-core input maps (host-side sharding/layout)."""
    xs = [np.asarray(x)[0, 0] for x in (x1, x2, x3, x4)]   # (4096, 66) each
    x1f = xs[0]
    wall = np.concatenate([np.asarray(wq), np.asarray(wk)], 0).astype(np.float32)
    ball = np.concatenate([np.asarray(bq), np.asarray(bk)], 0).astype(np.float32)
    in_maps = []
    for core in range(8):
        m, h = core // 2, core % 2
        xm = xs[m]
        xq = np.ascontiguousarray(x1f[2048 * h: 2048 * h + 2048, :].T)  # (66, 2048)
        # Local key buffer: local chunk lk = global chunk lk + 16*(h-1)
        xkbuf = np.zeros((KW, D), np.float32)
        vt = np.zeros((KW, 4), np.float32)
        vt[:, 2] = TBIG
        g0 = 16 * (h - 1)
        lo_l = max(0, -g0)
        hi_l = min(KCH, 32 - g0)
        gl0 = (lo_l + g0) * PC
        gl1 = (hi_l + g0) * PC
        xkbuf[lo_l * PC: hi_l * PC] = xm[gl0:gl1]
        vt[lo_l * PC: hi_l * PC, 0:2] = xm[gl0:gl1, 0:2]
        vt[lo_l * PC: hi_l * PC, 2] = xm[gl0:gl1, 65]
        vt_t = np.ascontiguousarray(
            vt.reshape(KCH, PC, 4).transpose(1, 0, 2).reshape(PC, KCH * 4))
        im = {
            "xq": xq.astype(np.float32),
            "xk": np.ascontiguousarray(xkbuf.T),
            "vt": vt_t,        # (128, 132) pre-tiled [v0 v1 t2 0] per chunk
            "t1r": np.ascontiguousarray(x1f[2048 * h: 2048 * h + 2048, 65][None, :]),
            "wall": np.ascontiguousarray(
                wall.transpose(1, 0, 2).reshape(D, 6 * D)),   # (66, 396) f-major
            "ball": np.ascontiguousarray(ball.T),             # (66, 6)
        }
        in_maps.append(im)
    return in_maps


def _window_ok(x1, xs):
    """Check the bounded-rank-deviation assumption the device program needs."""
    t1 = np.asarray(x1)[0, 0, :, 65]
    for xm in xs:
        t2 = np.asarray(xm)[0, 0, :, 65]
        p = np.searchsorted(t2, t1, side="right")
        b = np.arange(32)
        if not (p[b * PC] >= (b - 1) * PC).all():
            return False
        if not (p[b * PC + PC - 1] <= (b + 2) * PC).all():
            return False
    return True


def _core_emulate(im):
    """Numpy emulation of the device program for one core (validation)."""
    def mlp(x_T, ws, bs):
        h = x_T
        for l in range(3):
            h = ws[l].T @ h + bs[l][:, None]
            if l < 2:
                h = np.maximum(h, 0.0)
        return h

    ws = [im["wall"][:, i * D:(i + 1) * D] for i in range(6)]
    bs = [im["ball"][:, i] for i in range(6)]
    qT = mlp(im["xq"], ws[0:3], bs[0:3])                    # (66, 2048)
    kT = mlp(im["xk"], ws[3:6], bs[3:6])                    # (66, 4224)
    vt = im["vt"].reshape(PC, KCH, 4).transpose(1, 0, 2).reshape(KW, 4)
    V = vt[:, 0:2]
    t2 = vt[:, 2]
    t1 = im["t1r"][0]

    csum = np.zeros((KCH, D, 2), np.float32)
    for lk in range(KCH):
        csum[lk] = kT[:, lk * PC:(lk + 1) * PC] @ V[lk * PC:(lk + 1) * PC]
    out = np.zeros((QW, 2), np.float32)
    spre_s = np.zeros((QCH, D, 2), np.float32)
    acc = np.zeros((D, 2), np.float32)
    for lk in range(30):
        acc = acc + csum[lk]
        if 14 <= lk <= 29:
            spre_s[lk - 14] = acc
    for lb in range(QCH):
        qc = qT[:, lb * PC:(lb + 1) * PC]
        o = qc.T @ spre_s[lb]
        for w in range(3):
            lk = lb + 15 + w
            kc = kT[:, lk * PC:(lk + 1) * PC]
            sc = kc.T @ qc
            cmp = (t1[None, lb * PC:(lb + 1) * PC] >=
                   t2[lk * PC:(lk + 1) * PC, None]).astype(np.float32)
            o = o + (sc * cmp).T @ V[lk * PC:(lk + 1) * PC]
        out[lb * PC:(lb + 1) * PC] = o
    return out.T.copy()        # (2, 2048) like the device output


def _combine(per_core_outs):
    full = np.zeros((T, 2), np.float32)
    for core, o in enumerate(per_core_outs):
        h = core % 2
        full[2048 * h: 2048 * h + 2048] += o.T
    return full[None, :, :]


def _numpy_fallback(x1, x2, x3, x4, wq, bq, wk, bk):
    """Exact dense fallback (used only if the window assumption fails)."""
    xs = [np.asarray(x)[0, 0].astype(np.float64) for x in (x1, x2, x3, x4)]

    def mlp(x, W, b):
        h = x
        for l in range(2):
            h = np.maximum(h @ W[l] + b[l], 0.0)
        return h @ W[2] + b[2]

    Q = mlp(xs[0], np.asarray(wq, np.float64), np.asarray(bq, np.float64))
    t1 = xs[0][:, 65]
    out = np.zeros((T, 2))
    for m in range(M):
        Km = mlp(xs[m], np.asarray(wk, np.float64), np.asarray(bk, np.float64))
        t2 = xs[m][:, 65]
        mask = t2[None, :] <= t1[:, None]
        A = (Q @ Km.T) * mask
        out += A @ xs[m][:, 0:2]
    return out[None].astype(np.float32)


# ---------------------------------------------------------------------------
# Bass device program
# ---------------------------------------------------------------------------

_NC_CACHE = {}


def _build_nc():
    import concourse.bacc as bacc
    import concourse.mybir as mybir
    import concourse.tile as tile
    from concourse import masks

    f32 = mybir.dt.float32
    f32r = mybir.dt.float32r
    f16 = mybir.dt.float16
    AF = mybir.ActivationFunctionType
    ALU = mybir.AluOpType

    nc = bacc.Bacc("TRN2", target_bir_lowering=False, debug=False,
                   enable_asserts=False, num_devices=8)

    xq_d = nc.dram_tensor("xq", [D, QW], f32r, kind="ExternalInput")
    xk_d = nc.dram_tensor("xk", [D, KW], f32r, kind="ExternalInput")
    vt_d = nc.dram_tensor("vt", [PC, KCH * 4], f32, kind="ExternalInput")
    t1_d = nc.dram_tensor("t1r", [1, QW], f32, kind="ExternalInput")
    wall_d = nc.dram_tensor("wall", [D, 6 * D], f32r, kind="ExternalInput")
    ball_d = nc.dram_tensor("ball", [D, 6], f32, kind="ExternalInput")
    out_d = nc.dram_tensor("out", [2, QW], f32, kind="ExternalOutput")

    with tile.TileContext(nc) as tc:
        with (
            tc.tile_pool(name="const", bufs=1) as cpool,
            tc.tile_pool(name="big", bufs=1) as bpool,
            tc.tile_pool(name="mlp", bufs=2) as mpool,
            tc.tile_pool(name="work", bufs=5) as wpool,
            tc.tile_pool(name="ps_main", bufs=3, space="PSUM") as ps_main,
            tc.tile_pool(name="ps_out", bufs=1, space="PSUM") as ps_out,
            tc.tile_pool(name="ps_spre", bufs=1, space="PSUM") as ps_spre,
        ):
            # ---- constants (contiguous DMAs, host pre-packed)
            wsb = cpool.tile([D, 6 * D], f32r)
            nc.sync.dma_start(wsb[:], wall_d[:])
            bsb = cpool.tile([D, 6], f32)
            nc.sync.dma_start(bsb[:], ball_d[:])
            ident = cpool.tile([128, 128], f32)
            masks.make_identity(nc, ident[:])
            ident_r = cpool.tile([128, 128], f32r)
            nc.vector.tensor_copy(ident_r[:], ident[:])

            # ---- activations (block DMAs so compute starts early); spread
            #      dispatch across idle sequencers
            xq = bpool.tile([D, QW], f32r)
            xk = bpool.tile([D, KW], f32r)
            for c0 in range(0, QW, 1024):
                nc.sync.dma_start(xq[:, c0:c0 + 1024], xq_d[:, c0:c0 + 1024])
            for c0 in range(0, KW, 1024):
                cw = min(1024, KW - c0)
                nc.sync.dma_start(xk[:, c0:c0 + cw], xk_d[:, c0:c0 + cw])
            vtile = bpool.tile([128, KCH * 4], f32)
            nc.sync.dma_start(vtile[:], vt_d[:])
            v16 = bpool.tile([128, KCH * 2], f16)   # V in fp16 for AV matmuls
            nc.vector.tensor_copy(
                v16[:].rearrange("p (n c) -> p n c", c=2),
                vtile[:].rearrange("p (n c) -> p n c", c=4)[:, :, 0:2])
            t1b = bpool.tile([128, QW], f32)
            for c0 in range(0, QW, 1024):
                nc.sync.dma_start(t1b[:, c0:c0 + 1024],
                                  t1_d[:, c0:c0 + 1024].broadcast_to((128, 1024)))

            # ---- MLPs, Q/K emission interleaved per layer so independent
            #      blocks keep every engine fed
            qTr = bpool.tile([D, QW], f32r)
            kT = bpool.tile([D, KW], f32r)
            ktm = bpool.tile([128, 30 * D], f32)
            jobs = {"q": (xq, QW, 0, qTr), "k": (xk, KW, 3, kT)}
            cur = {nm: j[0] for nm, j in jobs.items()}
            eng = 0
            for l in range(3):
                nxt = {}
                for nm, (src0, width, wofs, outt) in jobs.items():
                    nxt[nm] = outt if l == 2 else mpool.tile(
                        [D, width], f32r, tag=f"h{nm}", name=f"h{nm}{l}")
                blocks = []
                for nm, (src0, width, wofs, outt) in jobs.items():
                    for c0 in range(0, width, 1024):
                        blocks.append((nm, c0, min(1024, width - c0)))
                # round-robin q/k blocks
                blocks.sort(key=lambda b: (b[0] != 'q', b[1]))
                for nm, c0, bw in blocks:
                    _, width, wofs, _ = jobs[nm]
                    w_ap = wsb[:, (wofs + l) * D:(wofs + l + 1) * D]
                    b_ap = bsb[:, wofs + l:wofs + l + 1]
                    ps = ps_main.tile([D, 1024], f32, tag="m",
                                      name=f"mlp{nm}{l}{c0}")
                    for s0 in range(0, bw, 512):
                        sw = min(512, bw - s0)
                        nc.tensor.matmul(ps[:, s0:s0 + sw], w_ap,
                                         cur[nm][:, c0 + s0:c0 + s0 + sw],
                                         start=True, stop=True)
                    dst = nxt[nm]
                    if l < 2:
                        if eng % 2 == 1:
                            nc.scalar.activation(dst[:, c0:c0 + bw], ps[:, :bw],
                                                 AF.Relu, bias=b_ap)
                        else:
                            nc.vector.tensor_scalar(dst[:, c0:c0 + bw], ps[:, :bw],
                                                    b_ap, 0.0, ALU.add, ALU.max)
                    else:
                        if eng % 2 == 1:
                            nc.scalar.activation(dst[:, c0:c0 + bw], ps[:, :bw],
                                                 AF.Identity, bias=b_ap)
                        else:
                            nc.vector.tensor_scalar_add(dst[:, c0:c0 + bw],
                                                        ps[:, :bw], b_ap)
                    eng += 1
                cur = nxt
            del cur

            # ---- K to t-major via PE transpose (prefix chunks only)
            for g0 in range(0, 30, 5):
                g1 = min(g0 + 5, 30)
                pst = ps_main.tile([128, 384], f32r, tag="m", name=f"tr{g0}")
                for j, lk in enumerate(range(g0, g1)):
                    nc.tensor.transpose(pst[:, j * D:(j + 1) * D],
                                        kT[:, lk * PC:(lk + 1) * PC],
                                        ident_r[:D, :D])
                nc.scalar.activation(ktm[:, g0 * D:g1 * D],
                                     pst[:, :(g1 - g0) * D], AF.Copy)

            # ---- scores + fused mask per key chunk (kT ready block by block)
            mscb = {}
            for lk in range(15, KCH):
                lb0 = max(0, lk - 17)
                lb1 = min(QCH - 1, lk - 15)
                ncol = (lb1 - lb0 + 1) * PC
                ps = ps_main.tile([128, 384], f32, tag="m", name=f"scb{lk}")
                nc.tensor.matmul(ps[:, :ncol], kT[:, lk * PC:(lk + 1) * PC],
                                 qTr[:, lb0 * PC:(lb1 + 1) * PC],
                                 start=True, stop=True)
                msc = wpool.tile([128, 384], f16, tag="msc", name=f"msc{lk}")
                nc.vector.scalar_tensor_tensor(
                    msc[:, :ncol],
                    t1b[:, lb0 * PC:(lb1 + 1) * PC],
                    vtile[:, lk * 4 + 2:lk * 4 + 3],
                    ps[:, :ncol],
                    ALU.is_ge, ALU.mult)
                mscb[lk] = (msc, lb0)

            # ---- prefix states (serial chain, exact f32 accumulate)
            spre = bpool.tile([D, QCH * 2], f32r)

            def csum_mm(ps, lk, start, stop):
                nc.tensor.matmul(ps[:], ktm[:, lk * D:(lk + 1) * D],
                                 vtile[:, lk * 4:lk * 4 + 2],
                                 start=start, stop=stop)

            sacc = ps_spre.tile([D, 2], f32, tag="spx")
            for lk in range(15):
                csum_mm(sacc, lk, lk == 0, lk == 14)
            nc.scalar.copy(spre[:, 0:2], sacc[:])
            for lb in range(1, QCH):
                lk = lb + 14
                sp = ps_spre.tile([D, 2], f32, tag="spx", name=f"spinc{lb}")
                nc.tensor.matmul(sp[:], ident_r[:D, :D],
                                 spre[:, 2 * (lb - 1):2 * lb],
                                 start=True, stop=False)
                csum_mm(sp, lk, False, True)
                nc.scalar.copy(spre[:, 2 * lb:2 * lb + 2], sp[:])

            # ---- output accumulation; out psum holds 4 query chunks per bank
            outT = bpool.tile([2, QW], f32)
            for qb in range(QCH // 4):
                op = ps_out.tile([2, 512], f32, tag="out", name=f"op{qb}")
                for li in range(4):
                    lb = qb * 4 + li
                    oslc = op[:, 128 * li:128 * (li + 1)]
                    nc.tensor.matmul(oslc, spre[:, 2 * lb:2 * lb + 2],
                                     qTr[:, lb * PC:(lb + 1) * PC],
                                     start=True, stop=False)
                    for w in range(3):
                        lk = lb + 15 + w
                        msc, lb0 = mscb[lk]
                        nc.tensor.matmul(
                            oslc, v16[:, lk * 2:lk * 2 + 2],
                            msc[:, (lb - lb0) * PC:(lb - lb0 + 1) * PC],
                            start=False, stop=(w == 2))
                nc.scalar.copy(outT[:, qb * 512:(qb + 1) * 512], op[:])
                nc.sync.dma_start(out_d[:, qb * 512:(qb + 1) * 512],
                                  outT[:, qb * 512:(qb + 1) * 512])

    nc.compile()
    return nc


def _get_nc():
    if "nc" not in _NC_CACHE:
        _NC_CACHE["nc"] = _build_nc()
    return _NC_CACHE["nc"]


def kernel(x1, x2, x3, x4, wq, bq, wk, bk):
    xs = (x1, x2, x3, x4)
    if not _window_ok(x1, xs):
        return _numpy_fallback(x1, x2, x3, x4, wq, bq, wk, bk)
    in_maps = _shard_host(x1, x2, x3, x4, wq, bq, wk, bk)
    from concourse.bass_utils import run_bass_kernel_spmd
    nc = _get_nc()
    res = run_bass_kernel_spmd(nc, in_maps, list(range(8)))
    return _combine([r["out"] for r in res.results])
